# revision 55
# baseline (speedup 1.0000x reference)
"""Multi-head attention (RMSNorm q/k, dense softmax) on 8 TRN2 NeuronCores.

Sharding: core c -> batch b = c//2, head-group g = c%2 (8 of 16 heads).
Each core computes a partial y.T = (Wo_g @ O_g.T) for its batch; the host
sums the two head-group partials per batch and transposes back.

Device dataflow (per core; layouts keep the contraction dim on SBUF
partitions; x and all weights are pre-transposed AND pre-converted to bf16
on the host so every DMA is contiguous and every matmul runs at full PE
rate; fp32 accumulation happens in PSUM):
  V    = x @ Wv_g.T              -> V_aug [t, 8, 65] bf16 (ones column yields
                                    softmax denominators during the PV matmul)
  Q.T  = Wq_g @ x.T, RMS-normed  -> bf16 [128 (2 heads x 64 dh), 2048 t]
  K.T  likewise
  S.T  = K.T-slices^T @ Q.T      -> PSUM [128 k-tok, 1024]: both heads of a
                                    pair land in different banks/row-groups
  P.T  = exp(S.T / 8)            -> bf16 SBUF (ScalarE; no max-subtraction:
                                    |logits| <= 8 because q,k are RMS-normed)
  O.T  = V_aug^T @ P.T           -> PSUM [65, q]; row 64 = sum(exp) = den
  O.T /= den   (reciprocal_approx_fast + GpSimd partition_broadcast)
  y.T  = Wo_g.T^T @ O.T          -> PSUM -> SBUF -> DRAM

Scheduling structure (the performance-critical part):
- all projections + RMS statistics run up front; a tc.no_sync_barrier()
  keeps their Ln/Exp ops (and activation-table loads) out of the attention
  phase, leaving ScalarE a pure exp stream (~92% occupancy at steady state);
- RMSNorm rinv = exp(-0.5*ln(ms/64+eps)) stays on one activation table set;
  per-head sums of squares and the qn_w-weighted broadcast are matmuls;
- attention is a flat 2-deep software pipeline over all (head-pair, q-chunk,
  k-tile) steps: per step k emit S.T(k), exp(k-1), PV(k-2), with a 4-deep
  P.T ring so the Tile scheduler's PE reordering never creates WAR stalls;
- softmax denominators ride the PV matmul via the V ones-column; the
  normalize runs off the PV critical path (copy frees the accumulator, the
  reciprocal+broadcast happen on DVE/GpSimd, the multiply is deferred into
  the next block);
- the output projection is interleaved into the last head-pair's blocks.
"""

import numpy as np
import ml_dtypes

B, N, D, H, Dh = 4, 2048, 1024, 16, 64
HPC = 8  # heads per core
GD = HPC * Dh  # 512 out-dims per core per projection
EPS = float(np.finfo(np.float32).eps)
NT = N // 128  # 16 token tiles
DT = D // 128  # 8 contraction tiles over D
KT = GD // 128  # 4 contraction tiles over the head-dim group

_NC_CACHE = {}


def _build_nc():
    import concourse.tile as tile
    from concourse import bacc, mybir

    f32 = mybir.dt.float32
    f32r = mybir.dt.float32r
    bf16 = mybir.dt.bfloat16
    AF = mybir.ActivationFunctionType

    nc = bacc.Bacc(None, target_bir_lowering=False)

    xT_e = nc.declare_dram_parameter("xT", [D, N], bf16, isOutput=False)
    wqT_e = nc.declare_dram_parameter("wqT", [D, GD], bf16, isOutput=False)
    wkT_e = nc.declare_dram_parameter("wkT", [D, GD], bf16, isOutput=False)
    wvT_e = nc.declare_dram_parameter("wvT", [D, GD], bf16, isOutput=False)
    woT_e = nc.declare_dram_parameter("woT", [GD, D], bf16, isOutput=False)
    qnw_e = nc.declare_dram_parameter("qnw", [1, Dh], f32, isOutput=False)
    knw_e = nc.declare_dram_parameter("knw", [1, Dh], f32, isOutput=False)
    out_e = nc.declare_dram_parameter("out", [D, N], f32, isOutput=True)

    with nc.allow_low_precision(reason="f32r rounding / bf16 PV+out path"), \
            tile.TileContext(nc) as tc:
        from contextlib import ExitStack

        with ExitStack() as ctx:
            ep = ctx.enter_context
            # distinct tile names are distinct tags; a tag gets `bufs` slots
            consts = ep(tc.tile_pool(name="consts", bufs=1))
            xpool = ep(tc.tile_pool(name="x", bufs=1))
            wqp = ep(tc.tile_pool(name="wq", bufs=1))
            wkp = ep(tc.tile_pool(name="wk", bufs=1))
            wop = ep(tc.tile_pool(name="wo", bufs=1))
            vpool = ep(tc.tile_pool(name="v", bufs=1))
            qknp = ep(tc.tile_pool(name="qkn", bufs=1))  # 4 parity-named tags
            ptp = ep(tc.tile_pool(name="pt", bufs=2))
            otp = ep(tc.tile_pool(name="ot", bufs=1))
            scratch = ep(tc.tile_pool(name="scr", bufs=2))  # oraw gets 4 below
            smallp = ep(tc.tile_pool(name="small", bufs=2))
            stp = ep(tc.tile_pool(name="st", bufs=2, space="PSUM"))
            pvp = ep(tc.tile_pool(name="pv", bufs=2, space="PSUM"))
            auxp = ep(tc.tile_pool(name="aux", bufs=2, space="PSUM"))

            # ---- constants ----
            selq_raw = consts.tile([2, 128], f32)  # row g: qn_w at cols 64g..
            selk_raw = consts.tile([2, 128], f32)
            nc.vector.memset(selq_raw[:], 0.0)
            nc.vector.memset(selk_raw[:], 0.0)
            nc.sync.dma_start(selq_raw[0:1, 0:64], qnw_e[:, :])
            nc.sync.dma_start(selq_raw[1:2, 64:128], qnw_e[:, :])
            nc.sync.dma_start(selk_raw[0:1, 0:64], knw_e[:, :])
            nc.sync.dma_start(selk_raw[1:2, 64:128], knw_e[:, :])
            selq = consts.tile([2, 128], bf16)
            selk = consts.tile([2, 128], bf16)
            nc.vector.tensor_copy(selq[:], selq_raw[:])
            nc.vector.tensor_copy(selk[:], selk_raw[:])
            gones = consts.tile([128, 2], bf16)  # 64-group indicator
            nc.vector.memset(gones[:], 0.0)
            nc.vector.memset(gones[0:64, 0:1], 1.0)
            nc.vector.memset(gones[64:128, 1:2], 1.0)
            ones64 = consts.tile([1, 64], bf16)
            nc.vector.memset(ones64[:], 1.0)
            epsb = consts.tile([128, 1], f32)
            nc.vector.memset(epsb[:], EPS)

            # ---- weight / activation loads ----
            xt = []
            for i in range(DT):
                t = xpool.tile([128, N], bf16, name=f"xt{i}")
                nc.sync.dma_start(t[:], xT_e[128 * i : 128 * (i + 1), :])
                xt.append(t)
            # ---- V projection -> V_aug bf16 [t-tile][128, HPC, Dh+1] ----
            # (wv DMAs issue before wq/wk/wo: the V projection is the first
            # consumer; wo isn't needed until the output projection)
            vsb = []
            with tc.tile_pool(name="wv", bufs=1) as wvp:
                wv = []
                for i in range(DT):
                    t = wvp.tile([128, GD], bf16, name=f"wv{i}")
                    nc.sync.dma_start(t[:], wvT_e[128 * i : 128 * (i + 1), :])
                    wv.append(t)
                wq, wk = [], []
                for i in range(DT):
                    t = wqp.tile([128, GD], bf16, name=f"wq{i}")
                    nc.sync.dma_start(t[:], wqT_e[128 * i : 128 * (i + 1), :])
                    wq.append(t)
                    t = wkp.tile([128, GD], bf16, name=f"wk{i}")
                    nc.sync.dma_start(t[:], wkT_e[128 * i : 128 * (i + 1), :])
                    wk.append(t)
                wo = []
                for i in range(KT):
                    t = wop.tile([128, D], bf16, name=f"wo{i}")
                    nc.sync.dma_start(t[:], woT_e[128 * i : 128 * (i + 1), :])
                    wo.append(t)
                for tt in range(NT):
                    vps = pvp.tile([128, 512], f32, name="pv")
                    for dt_ in range(DT):
                        nc.tensor.matmul(
                            vps[:],
                            xt[dt_][:, 128 * tt : 128 * (tt + 1)],
                            wv[dt_][:],
                            start=(dt_ == 0),
                            stop=(dt_ == DT - 1),
                        )
                    vt = vpool.tile([128, HPC, Dh + 1], bf16, name=f"v{tt}")
                    nc.vector.tensor_copy(
                        vt[:, :, 0:Dh], vps.rearrange("p (h d) -> p h d", h=HPC)
                    )
                    nc.vector.memset(vt[:, :, Dh : Dh + 1], 1.0)
                    vsb.append(vt)

            # ---- O.T accumulator tiles (row block hp, all heads) ----
            ot = [otp.tile([128, N], bf16, name=f"ot{i}") for i in range(KT)]

            def qk_proj_chunks(hp):
                """Return (q_dst, k_dst, [8 chunk emitters]) for head pair hp.

                Each emitter projects + RMS-normalizes one 512-token chunk of
                one side; layout [128 partitions = 2 heads x 64 dh, 2048 t],
                dtype f32r (rounded by the producing DVE ops).
                """
                dsts = [
                    qknp.tile([128, N], bf16, name=f"qkn{hp}_{side}")
                    for side in range(2)
                ]

                def make(side, c4, phase):
                    wmat, sel = ((wq, selq), (wk, selk))[side]
                    dst = dsts[side]
                    # park ln(ms) in the (still unused) O.T tile rows 0-3
                    lsl = ot[hp][32 * side : 32 * side + 2, 512 * c4 : 512 * (c4 + 1)]

                    def emit_a():
                        # project one 512-token chunk, square, group-sum, ln
                        # (st pool is idle during the projection phase)
                        qps = stp.tile([128, 512], f32, name=f"st{c4 % 2}", bufs=1)
                        for dt_ in range(DT):
                            nc.tensor.matmul(
                                qps[:],
                                wmat[dt_][:, 128 * hp : 128 * (hp + 1)],
                                xt[dt_][:, 512 * c4 : 512 * (c4 + 1)],
                                start=(dt_ == 0),
                                stop=(dt_ == DT - 1),
                            )
                        sl = dst[:, 512 * c4 : 512 * (c4 + 1)]
                        nc.vector.tensor_copy(sl, qps[:])
                        q2 = scratch.tile([128, 512], bf16, name="q2")
                        nc.vector.tensor_mul(q2[:], sl, sl)
                        # ms[g, t] = sum of squares within each 64-row head
                        msps = auxp.tile([128, 512], f32, name="aux")
                        nc.tensor.matmul(
                            msps[0:2, :], gones[:], q2[:], start=True, stop=True
                        )
                        nc.scalar.activation(
                            lsl, msps[0:2, :], AF.Ln, bias=epsb[0:2], scale=1.0 / Dh
                        )

                    def emit_b():
                        # rinv = exp(-0.5*ln), expand with qn_w folded into sel
                        rinv = scratch.tile([2, 512], bf16, name="rinv", bufs=4)
                        nc.scalar.activation(rinv[:], lsl, AF.Exp, scale=-0.5)
                        rexp = auxp.tile([128, 512], f32, name="aux")
                        nc.tensor.matmul(
                            rexp[:], sel[:], rinv[:], start=True, stop=True
                        )
                        sl = dst[:, 512 * c4 : 512 * (c4 + 1)]
                        nc.vector.tensor_mul(sl, sl, rexp[:])

                    return emit_a if phase == 0 else emit_b

                return dsts, [
                    make(s, c, ph) for s in range(2) for c in range(4)
                    for ph in range(2)
                ]

            qkn_all = []
            all_a, all_b = [], []
            for hp_ in range(4):
                dsts, chunks = qk_proj_chunks(hp_)
                qkn_all.append(dsts)
                all_a.extend(chunks[0::2])
                all_b.extend(chunks[1::2])
            for em in all_a:
                em()
            for em in all_b:
                em()
            # keep the projection/rms Ln+Exp ops (and their activation-table
            # loads) out of the attention phase's pure-exp ScalarE stream
            tc.no_sync_barrier()
            pending = []  # deferred drain tails (PE rde + DVE mul)

            def outproj(tch):
                for do in range(DT):
                    yps = auxp.tile([128, 512], f32, name="aux")
                    for kt_ in range(KT):
                        nc.tensor.matmul(
                            yps[:],
                            wo[kt_][:, 128 * do : 128 * (do + 1)],
                            ot[kt_][:, 512 * tch : 512 * (tch + 1)],
                            start=(kt_ == 0),
                            stop=(kt_ == KT - 1),
                        )
                    ysb = scratch.tile([128, 512], f32, name="q2")
                    nc.vector.tensor_copy(ysb[:], yps[:])
                    nc.sync.dma_start(
                        out_e[128 * do : 128 * (do + 1), 512 * tch : 512 * (tch + 1)],
                        ysb[:],
                    )

            def finish_block(pvs, hp, qc, last_pt):
                for side in range(2):
                    nc.tensor.matmul(
                        pvs[side][:],
                        vsb[NT - 1][:, 2 * hp + side, :],
                        last_pt[:, 512 * side : 512 * (side + 1)],
                        start=False,
                        stop=True,
                    )
                for side in range(2):
                    p0 = 64 * side
                    # free the pv slot quickly; normalize out of scratch
                    oraw = scratch.tile([Dh + 1, 512], f32, name="oraw", bufs=4)
                    nc.vector.tensor_copy(oraw[:], pvs[side][:])
                    den0 = smallp.tile([1, 512], f32, name="den0", bufs=4)
                    nc.vector.tensor_copy(den0[:], pvs[side][Dh : Dh + 1, :])
                    rdenf = smallp.tile([1, 512], f32, name="rdenf", bufs=4)
                    nc.vector.reciprocal_approx_fast(rdenf[:], den0[:])
                    rden = smallp.tile([1, 512], bf16, name="rden", bufs=4)
                    nc.vector.tensor_copy(rden[:], rdenf[:])
                    osl = ot[hp][p0 : p0 + 64, 512 * qc : 512 * (qc + 1)]

                    def fin(oraw=oraw, rden=rden, osl=osl):
                        rde = auxp.tile([64, 512], f32, name="aux")
                        nc.tensor.matmul(
                            rde[:], ones64[:], rden[:], start=True, stop=True
                        )
                        nc.vector.tensor_mul(osl, oraw[0:Dh, :], rde[:])

                    pending.append(fin)
            # flat 2-deep pipeline over all (block, k-tile) steps:
            # per step k emit S.T(k), exp(k-1), PV(k-2) so every engine's
            # in-order stream keeps one step of lookahead, including across
            # block boundaries
            steps = [
                (hp, qc, j) for hp in range(4) for qc in range(4)
                for j in range(NT)
            ]
            n = len(steps)
            blk_pvs = {}
            sts = {}
            pts = {}

            def emit_st(k):
                hp, qc, j = steps[k]
                qn, kn = qkn_all[hp]
                st = stp.tile([128, 1024], f32, name=f"st{k % 2}", bufs=1)
                for side in range(2):
                    p0 = 64 * side
                    nc.tensor.matmul(
                        st[:, 512 * side : 512 * (side + 1)],
                        kn[p0 : p0 + 64, 128 * j : 128 * (j + 1)],
                        qn[p0 : p0 + 64, 512 * qc : 512 * (qc + 1)],
                        start=True,
                        stop=True,
                    )
                sts[k] = st

            def emit_exp(k):
                pt = ptp.tile([128, 1024], bf16, name=f"pt{k % 6}", bufs=1)
                nc.scalar.activation(pt[:], sts.pop(k)[:], AF.Exp, scale=Dh**-0.5)
                pts[k] = pt

            def emit_pv(k):
                hp, qc, j = steps[k]
                if j == 0:
                    blk_pvs[(hp, qc)] = [
                        pvp.tile([Dh + 1, 512], f32, name="pv") for _ in range(2)
                    ]
                pvs = blk_pvs[(hp, qc)]
                pt = pts.pop(k)
                for side in range(2):
                    nc.tensor.matmul(
                        pvs[side][:],
                        vsb[j][:, 2 * hp + side, :],
                        pt[:, 512 * side : 512 * (side + 1)],
                        start=(j == 0),
                        stop=(j == NT - 1),
                    )
                if j == NT - 1:
                    drain_block(pvs, hp, qc)
                if j == 6 and pending:
                    # two-blocks-ago normalization: its reciprocal (DVE) is
                    # long done, so the rde matmul doesn't stall PE
                    for fn in pending:
                        fn()
                    pending.clear()
                    if hp == 3 and qc > 0:
                        outproj(qc - 1)

            def drain_block(pvs, hp, qc):
                for side in range(2):
                    p0 = 64 * side
                    # free the pv slot quickly; normalize out of scratch
                    oraw = scratch.tile([Dh + 1, 512], f32, name="oraw", bufs=4)
                    nc.vector.tensor_copy(oraw[:], pvs[side][:])
                    den0 = smallp.tile([1, 512], f32, name="den0", bufs=4)
                    nc.vector.tensor_copy(den0[:], pvs[side][Dh : Dh + 1, :])
                    rdenf = smallp.tile([1, 512], f32, name="rdenf", bufs=4)
                    nc.vector.reciprocal_approx_fast(rdenf[:], den0[:])
                    # expand 1/den across the 64 dh rows on the idle GpSimd
                    rde = scratch.tile([Dh, 512], f32, name="rde", bufs=4)
                    nc.gpsimd.partition_broadcast(rde[:], rdenf[:], channels=Dh)
                    osl = ot[hp][p0 : p0 + 64, 512 * qc : 512 * (qc + 1)]

                    def fin(oraw=oraw, rde=rde, osl=osl):
                        nc.vector.tensor_mul(osl, oraw[0:Dh, :], rde[:])

                    pending.append(fin)


            for k in range(n + 2):
                if k < n:
                    emit_st(k)
                if 0 < k <= n:
                    emit_exp(k - 1)
                if 1 < k <= n + 1:
                    emit_pv(k - 2)
            for fn in pending:
                fn()
            outproj(3)

    nc.compile()
    return nc


def _get_nc():
    if "nc" not in _NC_CACHE:
        _NC_CACHE["nc"] = _build_nc()
    return _NC_CACHE["nc"]


def make_in_maps(x, Wq, Wk, Wv, Wo, qn_w, kn_w):
    x = np.asarray(x, np.float32)
    Wq, Wk, Wv, Wo = (np.asarray(w, np.float32) for w in (Wq, Wk, Wv, Wo))
    qn_w = np.asarray(qn_w, np.float32).reshape(1, Dh)
    kn_w = np.asarray(kn_w, np.float32).reshape(1, Dh)
    in_maps = []
    for c in range(8):
        b, g = c // 2, c % 2
        sl = slice(GD * g, GD * (g + 1))
        in_maps.append(
            {
                "xT": np.ascontiguousarray(x[b].T).astype(ml_dtypes.bfloat16),
                "wqT": np.ascontiguousarray(Wq[sl, :].T).astype(ml_dtypes.bfloat16),
                "wkT": np.ascontiguousarray(Wk[sl, :].T).astype(ml_dtypes.bfloat16),
                "wvT": np.ascontiguousarray(Wv[sl, :].T).astype(ml_dtypes.bfloat16),
                "woT": np.ascontiguousarray(Wo[:, sl].T).astype(ml_dtypes.bfloat16),
                "qnw": qn_w,
                "knw": kn_w,
            }
        )
    return in_maps


def assemble(results):
    out = np.empty((B, N, D), np.float32)
    for b in range(B):
        out[b] = (
            results[2 * b]["out"].astype(np.float32)
            + results[2 * b + 1]["out"].astype(np.float32)
        ).T
    return out


def kernel(x, Wq, Wk, Wv, Wo, qn_w, kn_w):
    from concourse.bass_utils import run_bass_kernel_spmd

    nc = _get_nc()
    in_maps = make_in_maps(x, Wq, Wk, Wv, Wo, qn_w, kn_w)
    res = run_bass_kernel_spmd(nc, in_maps, core_ids=list(range(8)))
    return assemble(res.results)


# revision 56
# speedup vs baseline: 1.0085x; 1.0085x over previous
"""Multi-head attention (RMSNorm q/k, dense softmax) on 8 TRN2 NeuronCores.

Sharding: core c -> batch b = c//2, head-group g = c%2 (8 of 16 heads).
Each core computes a partial y.T = (Wo_g @ O_g.T) for its batch; the host
sums the two head-group partials per batch and transposes back.

Device dataflow (per core; layouts keep the contraction dim on SBUF
partitions; x and all weights are pre-transposed AND pre-converted to bf16
on the host so every DMA is contiguous and every matmul runs at full PE
rate; fp32 accumulation happens in PSUM):
  V    = x @ Wv_g.T              -> V_aug [t, 8, 65] bf16 (ones column yields
                                    softmax denominators during the PV matmul)
  Q.T  = Wq_g @ x.T, RMS-normed  -> bf16 [128 (2 heads x 64 dh), 2048 t]
  K.T  likewise
  S.T  = K.T-slices^T @ Q.T      -> PSUM [128 k-tok, 1024]: both heads of a
                                    pair land in different banks/row-groups
  P.T  = exp(S.T / 8)            -> bf16 SBUF (ScalarE; no max-subtraction:
                                    |logits| <= 8 because q,k are RMS-normed)
  O.T  = V_aug^T @ P.T           -> PSUM [65, q]; row 64 = sum(exp) = den
  O.T /= den   (reciprocal_approx_fast + GpSimd partition_broadcast)
  y.T  = Wo_g.T^T @ O.T          -> PSUM -> SBUF -> DRAM

Scheduling structure (the performance-critical part):
- all projections + RMS statistics run up front; a tc.no_sync_barrier()
  keeps their Ln/Exp ops (and activation-table loads) out of the attention
  phase, leaving ScalarE a pure exp stream (~92% occupancy at steady state);
- RMSNorm rinv = exp(-0.5*ln(ms/64+eps)) stays on one activation table set;
  per-head sums of squares and the qn_w-weighted broadcast are matmuls;
- attention is a flat 2-deep software pipeline over all (head-pair, q-chunk,
  k-tile) steps: per step k emit S.T(k), exp(k-1), PV(k-2), with a 4-deep
  P.T ring so the Tile scheduler's PE reordering never creates WAR stalls;
- softmax denominators ride the PV matmul via the V ones-column; the
  normalize runs off the PV critical path (copy frees the accumulator, the
  reciprocal+broadcast happen on DVE/GpSimd, the multiply is deferred into
  the next block);
- the output projection is interleaved into the last head-pair's blocks.
"""

import numpy as np
import ml_dtypes

B, N, D, H, Dh = 4, 2048, 1024, 16, 64
HPC = 8  # heads per core
GD = HPC * Dh  # 512 out-dims per core per projection
EPS = float(np.finfo(np.float32).eps)
NT = N // 128  # 16 token tiles
DT = D // 128  # 8 contraction tiles over D
KT = GD // 128  # 4 contraction tiles over the head-dim group

_NC_CACHE = {}


def _build_nc():
    import concourse.tile as tile
    from concourse import bacc, mybir

    f32 = mybir.dt.float32
    f32r = mybir.dt.float32r
    bf16 = mybir.dt.bfloat16
    AF = mybir.ActivationFunctionType

    nc = bacc.Bacc(None, target_bir_lowering=False)

    xT_e = nc.declare_dram_parameter("xT", [D, N], bf16, isOutput=False)
    wqT_e = nc.declare_dram_parameter("wqT", [D, GD], bf16, isOutput=False)
    wkT_e = nc.declare_dram_parameter("wkT", [D, GD], bf16, isOutput=False)
    wvT_e = nc.declare_dram_parameter("wvT", [D, GD], bf16, isOutput=False)
    woT_e = nc.declare_dram_parameter("woT", [GD, D], bf16, isOutput=False)
    qnw_e = nc.declare_dram_parameter("qnw", [1, Dh], f32, isOutput=False)
    knw_e = nc.declare_dram_parameter("knw", [1, Dh], f32, isOutput=False)
    out_e = nc.declare_dram_parameter("out", [D, N], f32, isOutput=True)

    with nc.allow_low_precision(reason="f32r rounding / bf16 PV+out path"), \
            tile.TileContext(nc) as tc:
        from contextlib import ExitStack

        with ExitStack() as ctx:
            ep = ctx.enter_context
            # distinct tile names are distinct tags; a tag gets `bufs` slots
            consts = ep(tc.tile_pool(name="consts", bufs=1))
            xpool = ep(tc.tile_pool(name="x", bufs=1))
            wqp = ep(tc.tile_pool(name="wq", bufs=1))
            wkp = ep(tc.tile_pool(name="wk", bufs=1))
            wop = ep(tc.tile_pool(name="wo", bufs=1))
            vpool = ep(tc.tile_pool(name="v", bufs=1))
            qknp = ep(tc.tile_pool(name="qkn", bufs=1))  # 4 parity-named tags
            ptp = ep(tc.tile_pool(name="pt", bufs=2))
            otp = ep(tc.tile_pool(name="ot", bufs=1))
            scratch = ep(tc.tile_pool(name="scr", bufs=2))  # oraw gets 4 below
            smallp = ep(tc.tile_pool(name="small", bufs=2))
            stp = ep(tc.tile_pool(name="st", bufs=2, space="PSUM"))
            pvp = ep(tc.tile_pool(name="pv", bufs=2, space="PSUM"))
            auxp = ep(tc.tile_pool(name="aux", bufs=2, space="PSUM"))

            # ---- constants ----
            selq_raw = consts.tile([2, 128], f32)  # row g: qn_w at cols 64g..
            selk_raw = consts.tile([2, 128], f32)
            nc.vector.memset(selq_raw[:], 0.0)
            nc.vector.memset(selk_raw[:], 0.0)
            nc.sync.dma_start(selq_raw[0:1, 0:64], qnw_e[:, :])
            nc.sync.dma_start(selq_raw[1:2, 64:128], qnw_e[:, :])
            nc.sync.dma_start(selk_raw[0:1, 0:64], knw_e[:, :])
            nc.sync.dma_start(selk_raw[1:2, 64:128], knw_e[:, :])
            selq = consts.tile([2, 128], bf16)
            selk = consts.tile([2, 128], bf16)
            nc.vector.tensor_copy(selq[:], selq_raw[:])
            nc.vector.tensor_copy(selk[:], selk_raw[:])
            gones = consts.tile([128, 2], bf16)  # 64-group indicator
            nc.vector.memset(gones[:], 0.0)
            nc.vector.memset(gones[0:64, 0:1], 1.0)
            nc.vector.memset(gones[64:128, 1:2], 1.0)
            ones64 = consts.tile([1, 64], bf16)
            nc.vector.memset(ones64[:], 1.0)
            epsb = consts.tile([128, 1], f32)
            nc.vector.memset(epsb[:], EPS)

            # ---- weight / activation loads ----
            xt = []
            for i in range(DT):
                t = xpool.tile([128, N], bf16, name=f"xt{i}")
                nc.sync.dma_start(t[:], xT_e[128 * i : 128 * (i + 1), :])
                xt.append(t)
            # ---- V projection -> V_aug bf16 [t-tile][128, HPC, Dh+1] ----
            # (wv DMAs issue before wq/wk/wo: the V projection is the first
            # consumer; wo isn't needed until the output projection)
            vsb = []
            with tc.tile_pool(name="wv", bufs=1) as wvp:
                wv = []
                for i in range(DT):
                    t = wvp.tile([128, GD], bf16, name=f"wv{i}")
                    nc.sync.dma_start(t[:], wvT_e[128 * i : 128 * (i + 1), :])
                    wv.append(t)
                wq, wk = [], []
                for i in range(DT):
                    t = wqp.tile([128, GD], bf16, name=f"wq{i}")
                    nc.sync.dma_start(t[:], wqT_e[128 * i : 128 * (i + 1), :])
                    wq.append(t)
                    t = wkp.tile([128, GD], bf16, name=f"wk{i}")
                    nc.sync.dma_start(t[:], wkT_e[128 * i : 128 * (i + 1), :])
                    wk.append(t)
                wo = []
                for i in range(KT):
                    t = wop.tile([128, D], bf16, name=f"wo{i}")
                    nc.sync.dma_start(t[:], woT_e[128 * i : 128 * (i + 1), :])
                    wo.append(t)
                for tt in range(NT):
                    vps = pvp.tile([128, 512], f32, name="pv")
                    for dt_ in range(DT):
                        nc.tensor.matmul(
                            vps[:],
                            xt[dt_][:, 128 * tt : 128 * (tt + 1)],
                            wv[dt_][:],
                            start=(dt_ == 0),
                            stop=(dt_ == DT - 1),
                        )
                    vt = vpool.tile([128, HPC, Dh + 1], bf16, name=f"v{tt}")
                    nc.vector.tensor_copy(
                        vt[:, :, 0:Dh], vps.rearrange("p (h d) -> p h d", h=HPC)
                    )
                    nc.vector.memset(vt[:, :, Dh : Dh + 1], 1.0)
                    vsb.append(vt)

            # ---- O.T accumulator tiles (row block hp, all heads) ----
            ot = [otp.tile([128, N], bf16, name=f"ot{i}") for i in range(KT)]

            def qk_proj_chunks(hp):
                """Return (q_dst, k_dst, [8 chunk emitters]) for head pair hp.

                Each emitter projects + RMS-normalizes one 512-token chunk of
                one side; layout [128 partitions = 2 heads x 64 dh, 2048 t],
                dtype f32r (rounded by the producing DVE ops).
                """
                dsts = [
                    qknp.tile([128, N], bf16, name=f"qkn{hp}_{side}")
                    for side in range(2)
                ]

                def make(side, c4, phase):
                    wmat, sel = ((wq, selq), (wk, selk))[side]
                    dst = dsts[side]
                    # park ln(ms) in the (still unused) O.T tile rows 0-3
                    lsl = ot[hp][32 * side : 32 * side + 2, 512 * c4 : 512 * (c4 + 1)]

                    def emit_a():
                        # project one 512-token chunk, square, group-sum, ln
                        # (st pool is idle during the projection phase)
                        qps = stp.tile([128, 512], f32, name=f"st{c4 % 2}", bufs=1)
                        for dt_ in range(DT):
                            nc.tensor.matmul(
                                qps[:],
                                wmat[dt_][:, 128 * hp : 128 * (hp + 1)],
                                xt[dt_][:, 512 * c4 : 512 * (c4 + 1)],
                                start=(dt_ == 0),
                                stop=(dt_ == DT - 1),
                            )
                        sl = dst[:, 512 * c4 : 512 * (c4 + 1)]
                        nc.vector.tensor_copy(sl, qps[:])
                        q2 = scratch.tile([128, 512], bf16, name="q2")
                        nc.vector.tensor_mul(q2[:], sl, sl)
                        # ms[g, t] = sum of squares within each 64-row head
                        msps = auxp.tile([128, 512], f32, name="aux")
                        nc.tensor.matmul(
                            msps[0:2, :], gones[:], q2[:], start=True, stop=True
                        )
                        nc.scalar.activation(
                            lsl, msps[0:2, :], AF.Ln, bias=epsb[0:2], scale=1.0 / Dh
                        )

                    def emit_b():
                        # rinv = exp(-0.5*ln), expand with qn_w folded into sel
                        rinv = scratch.tile([2, 512], bf16, name="rinv", bufs=4)
                        nc.scalar.activation(rinv[:], lsl, AF.Exp, scale=-0.5)
                        rexp = auxp.tile([128, 512], f32, name="aux")
                        nc.tensor.matmul(
                            rexp[:], sel[:], rinv[:], start=True, stop=True
                        )
                        sl = dst[:, 512 * c4 : 512 * (c4 + 1)]
                        nc.vector.tensor_mul(sl, sl, rexp[:])

                    return emit_a if phase == 0 else emit_b

                return dsts, [
                    make(s, c, ph) for s in range(2) for c in range(4)
                    for ph in range(2)
                ]

            qkn_all = []
            all_a, all_b = [], []
            for hp_ in range(4):
                dsts, chunks = qk_proj_chunks(hp_)
                qkn_all.append(dsts)
                all_a.extend(chunks[0::2])
                all_b.extend(chunks[1::2])
            for em in all_a:
                em()
            for em in all_b:
                em()
            # keep the projection/rms Ln+Exp ops (and their activation-table
            # loads) out of the attention phase's pure-exp ScalarE stream
            tc.no_sync_barrier()
            pending = []  # deferred drain tails (PE rde + DVE mul)

            def outproj(tch, dos=None):
                for do in (range(DT) if dos is None else dos):
                    yps = auxp.tile([128, 512], f32, name="aux")
                    for kt_ in range(KT):
                        nc.tensor.matmul(
                            yps[:],
                            wo[kt_][:, 128 * do : 128 * (do + 1)],
                            ot[kt_][:, 512 * tch : 512 * (tch + 1)],
                            start=(kt_ == 0),
                            stop=(kt_ == KT - 1),
                        )
                    ysb = scratch.tile([128, 512], f32, name="q2")
                    nc.vector.tensor_copy(ysb[:], yps[:])
                    nc.sync.dma_start(
                        out_e[128 * do : 128 * (do + 1), 512 * tch : 512 * (tch + 1)],
                        ysb[:],
                    )

            def finish_block(pvs, hp, qc, last_pt):
                for side in range(2):
                    nc.tensor.matmul(
                        pvs[side][:],
                        vsb[NT - 1][:, 2 * hp + side, :],
                        last_pt[:, 512 * side : 512 * (side + 1)],
                        start=False,
                        stop=True,
                    )
                for side in range(2):
                    p0 = 64 * side
                    # free the pv slot quickly; normalize out of scratch
                    oraw = scratch.tile([Dh + 1, 512], f32, name="oraw", bufs=4)
                    nc.vector.tensor_copy(oraw[:], pvs[side][:])
                    den0 = smallp.tile([1, 512], f32, name="den0", bufs=4)
                    nc.vector.tensor_copy(den0[:], pvs[side][Dh : Dh + 1, :])
                    rdenf = smallp.tile([1, 512], f32, name="rdenf", bufs=4)
                    nc.vector.reciprocal_approx_fast(rdenf[:], den0[:])
                    rden = smallp.tile([1, 512], bf16, name="rden", bufs=4)
                    nc.vector.tensor_copy(rden[:], rdenf[:])
                    osl = ot[hp][p0 : p0 + 64, 512 * qc : 512 * (qc + 1)]

                    def fin(oraw=oraw, rden=rden, osl=osl):
                        rde = auxp.tile([64, 512], f32, name="aux")
                        nc.tensor.matmul(
                            rde[:], ones64[:], rden[:], start=True, stop=True
                        )
                        nc.vector.tensor_mul(osl, oraw[0:Dh, :], rde[:])

                    pending.append(fin)
            # flat 2-deep pipeline over all (block, k-tile) steps:
            # per step k emit S.T(k), exp(k-1), PV(k-2) so every engine's
            # in-order stream keeps one step of lookahead, including across
            # block boundaries
            steps = [
                (hp, qc, j) for hp in range(4) for qc in range(4)
                for j in range(NT)
            ]
            n = len(steps)
            blk_pvs = {}
            sts = {}
            pts = {}

            def emit_st(k):
                hp, qc, j = steps[k]
                qn, kn = qkn_all[hp]
                st = stp.tile([128, 1024], f32, name=f"st{k % 2}", bufs=1)
                for side in range(2):
                    p0 = 64 * side
                    nc.tensor.matmul(
                        st[:, 512 * side : 512 * (side + 1)],
                        kn[p0 : p0 + 64, 128 * j : 128 * (j + 1)],
                        qn[p0 : p0 + 64, 512 * qc : 512 * (qc + 1)],
                        start=True,
                        stop=True,
                    )
                sts[k] = st

            def emit_exp(k):
                pt = ptp.tile([128, 1024], bf16, name=f"pt{k % 6}", bufs=1)
                nc.scalar.activation(pt[:], sts.pop(k)[:], AF.Exp, scale=Dh**-0.5)
                pts[k] = pt

            def emit_pv(k):
                hp, qc, j = steps[k]
                if j == 0:
                    blk_pvs[(hp, qc)] = [
                        pvp.tile([Dh + 1, 512], f32, name="pv") for _ in range(2)
                    ]
                pvs = blk_pvs[(hp, qc)]
                pt = pts.pop(k)
                for side in range(2):
                    nc.tensor.matmul(
                        pvs[side][:],
                        vsb[j][:, 2 * hp + side, :],
                        pt[:, 512 * side : 512 * (side + 1)],
                        start=(j == 0),
                        stop=(j == NT - 1),
                    )
                if j == NT - 1:
                    drain_block(pvs, hp, qc)
                if j == 6 and pending:
                    # two-blocks-ago normalization: its reciprocal (DVE) is
                    # long done, so the rde matmul doesn't stall PE
                    for fn in pending:
                        fn()
                    pending.clear()
                # interleave the previous token-chunk's output projection in
                # four small pieces so its PE bursts don't stall the exp
                # cadence (fins for chunk qc-1 are flushed by j == 6)
                if hp == 3 and qc > 0 and j in (6, 9, 12, 15):
                    i4 = (6, 9, 12, 15).index(j)
                    outproj(qc - 1, dos=range(2 * i4, 2 * i4 + 2))

            def drain_block(pvs, hp, qc):
                for side in range(2):
                    p0 = 64 * side
                    # free the pv slot quickly; normalize out of scratch
                    oraw = scratch.tile([Dh + 1, 512], f32, name="oraw", bufs=4)
                    nc.vector.tensor_copy(oraw[:], pvs[side][:])
                    den0 = smallp.tile([1, 512], f32, name="den0", bufs=4)
                    nc.vector.tensor_copy(den0[:], pvs[side][Dh : Dh + 1, :])
                    rdenf = smallp.tile([1, 512], f32, name="rdenf", bufs=4)
                    nc.vector.reciprocal_approx_fast(rdenf[:], den0[:])
                    # expand 1/den across the 64 dh rows on the idle GpSimd
                    rde = scratch.tile([Dh, 512], f32, name="rde", bufs=4)
                    nc.gpsimd.partition_broadcast(rde[:], rdenf[:], channels=Dh)
                    osl = ot[hp][p0 : p0 + 64, 512 * qc : 512 * (qc + 1)]

                    def fin(oraw=oraw, rde=rde, osl=osl):
                        nc.vector.tensor_mul(osl, oraw[0:Dh, :], rde[:])

                    pending.append(fin)


            for k in range(n + 2):
                if k < n:
                    emit_st(k)
                if 0 < k <= n:
                    emit_exp(k - 1)
                if 1 < k <= n + 1:
                    emit_pv(k - 2)
            for fn in pending:
                fn()
            outproj(3)

    nc.compile()
    return nc


def _get_nc():
    if "nc" not in _NC_CACHE:
        _NC_CACHE["nc"] = _build_nc()
    return _NC_CACHE["nc"]


def make_in_maps(x, Wq, Wk, Wv, Wo, qn_w, kn_w):
    x = np.asarray(x, np.float32)
    Wq, Wk, Wv, Wo = (np.asarray(w, np.float32) for w in (Wq, Wk, Wv, Wo))
    qn_w = np.asarray(qn_w, np.float32).reshape(1, Dh)
    kn_w = np.asarray(kn_w, np.float32).reshape(1, Dh)
    in_maps = []
    for c in range(8):
        b, g = c // 2, c % 2
        sl = slice(GD * g, GD * (g + 1))
        in_maps.append(
            {
                "xT": np.ascontiguousarray(x[b].T).astype(ml_dtypes.bfloat16),
                "wqT": np.ascontiguousarray(Wq[sl, :].T).astype(ml_dtypes.bfloat16),
                "wkT": np.ascontiguousarray(Wk[sl, :].T).astype(ml_dtypes.bfloat16),
                "wvT": np.ascontiguousarray(Wv[sl, :].T).astype(ml_dtypes.bfloat16),
                "woT": np.ascontiguousarray(Wo[:, sl].T).astype(ml_dtypes.bfloat16),
                "qnw": qn_w,
                "knw": kn_w,
            }
        )
    return in_maps


def assemble(results):
    out = np.empty((B, N, D), np.float32)
    for b in range(B):
        out[b] = (
            results[2 * b]["out"].astype(np.float32)
            + results[2 * b + 1]["out"].astype(np.float32)
        ).T
    return out


def kernel(x, Wq, Wk, Wv, Wo, qn_w, kn_w):
    from concourse.bass_utils import run_bass_kernel_spmd

    nc = _get_nc()
    in_maps = make_in_maps(x, Wq, Wk, Wv, Wo, qn_w, kn_w)
    res = run_bass_kernel_spmd(nc, in_maps, core_ids=list(range(8)))
    return assemble(res.results)
